# revision 3
# baseline (speedup 1.0000x reference)
"""MoE router GEMM on 8 TRN2 NeuronCores.

logits[t, e] = sum_d x[t, d] * w[e, d]
  x: [16384, 6144] bf16, w: [768, 6144] bf16, out fp32 [16384, 768].

Sharding: tokens split 8 ways (data parallel), weight replicated; each core
computes its [2048, 768] logits shard; host concatenates (the "all-gather").

Per-core kernel: w.T staged once into SBUF as [128 ki, 48 ko, 768 e] via XBAR
DMA-transpose (streaming matmul operand, resident); x staged in [128 ki, 48 ko,
M_TILE t] XBAR-transposed tiles (stationary operand); PSUM accumulates over the
48 k-subtiles for each 128-token row block; DVE evicts PSUM->SBUF; DMA out.
"""

import sys

for _p in ("/opt/trn_rl_repo", "/root/.axon_site/_ro/trn_rl_repo"):
    if _p not in sys.path:
        sys.path.insert(0, _p)

import numpy as np

N_CORES = 8
T_FULL = 16384
T = T_FULL // N_CORES  # 2048 tokens per core
D = 6144
E = 768
P = 128
KO = D // P  # 48 k-subtiles

_NC_CACHE = {}


def _build_nc(reps=1, m_tile=512, xbufs=2, obufs=3, psum_bufs=2, n_split=512):
    import concourse.bacc as bacc
    import concourse.mybir as mybir
    import concourse.tile as tile

    nc = bacc.Bacc("TRN2", target_bir_lowering=False, debug=False, num_devices=N_CORES)

    x = nc.dram_tensor("hidden_states", [T, D], mybir.dt.bfloat16, kind="ExternalInput")
    w = nc.dram_tensor("weight", [E, D], mybir.dt.bfloat16, kind="ExternalInput")
    out = nc.dram_tensor("out", [T, E], mybir.dt.float32, kind="ExternalOutput")

    with tile.TileContext(nc) as tc:
        if reps == 0:
            # null kernel: one tiny DMA roundtrip, for launch-overhead measurement
            with tc.tile_pool(name="null", bufs=1) as pool:
                t_in = pool.tile([P, 256], mybir.dt.bfloat16)
                nc.sync.dma_start(t_in[:], w[0:P, 0:256])
                t_out = pool.tile([P, 256], mybir.dt.float32)
                nc.vector.tensor_copy(t_out[:], t_in[:])
                nc.sync.dma_start(out[0:P, 0:256], t_out[:])
            nc.compile()
            return nc
        with (
            tc.tile_pool(name="wpool", bufs=1) as wpool,
            tc.tile_pool(name="xpool", bufs=xbufs) as xpool,
            tc.tile_pool(name="opool", bufs=obufs) as opool,
            tc.tile_pool(name="psum", bufs=psum_bufs, space="PSUM") as psum_pool,
        ):
            # w.T resident: wt[ki, ko, e] = w[e, ko*128 + ki]
            wt = wpool.tile([P, KO, E], mybir.dt.bfloat16)
            nc.sync.dma_start_transpose(
                wt[:], w.rearrange("e (ko ki) -> e ko ki", ki=P)
            )

            xv = x.rearrange("t (ko ki) -> t ko ki", ki=P)
            for rep in range(reps):
                for mt in range(T // m_tile):
                    # xt[ki, ko, t] = x[mt*m_tile + t, ko*128 + ki]
                    xt = xpool.tile([P, KO, m_tile], mybir.dt.bfloat16, tag="xt")
                    nc.sync.dma_start_transpose(
                        xt[:], xv[mt * m_tile : (mt + 1) * m_tile]
                    )
                    for ms in range(m_tile // P):
                        ptile = psum_pool.tile([P, E], mybir.dt.float32, tag="ps")
                        ot = opool.tile([P, E], mybir.dt.float32, tag="ot")
                        lhs = xt[:, :, ms * P : (ms + 1) * P]
                        for n0 in range(0, E, n_split):
                            n1 = min(n0 + n_split, E)
                            for ks in range(KO):
                                nc.tensor.matmul(
                                    ptile[:, n0:n1],
                                    lhs[:, ks],
                                    wt[:, ks, n0:n1],
                                    start=(ks == 0),
                                    stop=(ks == KO - 1),
                                )
                        nc.vector.tensor_copy(ot[:], ptile[:])
                        r0 = mt * m_tile + ms * P
                        nc.sync.dma_start(out[r0 : r0 + P, :], ot[:])

    nc.compile()
    return nc


def _get_nc(**kw):
    key = tuple(sorted(kw.items()))
    if key not in _NC_CACHE:
        _NC_CACHE[key] = _build_nc(**kw)
    return _NC_CACHE[key]


def _to_bf16_shards(hidden_states, weight):
    import ml_dtypes

    x = np.asarray(hidden_states)
    w = np.asarray(weight)
    if x.dtype != ml_dtypes.bfloat16:
        x = x.astype(ml_dtypes.bfloat16)
    if w.dtype != ml_dtypes.bfloat16:
        w = w.astype(ml_dtypes.bfloat16)
    assert x.shape == (T_FULL, D) and w.shape == (E, D)
    return [
        {"hidden_states": np.ascontiguousarray(x[i * T : (i + 1) * T]), "weight": w}
        for i in range(N_CORES)
    ]


def run_sharded(hidden_states, weight, trace=False, **build_kw):
    """Returns (out [16384, 768] fp32, BassKernelResults)."""
    from concourse.bass_utils import run_bass_kernel_spmd

    nc = _get_nc(**build_kw)
    in_maps = _to_bf16_shards(hidden_states, weight)
    res = run_bass_kernel_spmd(nc, in_maps, core_ids=list(range(N_CORES)), trace=trace)
    out = np.concatenate(
        [res.results[i]["out"] for i in range(N_CORES)], axis=0
    ).astype(np.float32, copy=False)
    return out, res


def kernel(hidden_states, weight):
    out, _ = run_sharded(hidden_states, weight, trace=False)
    return out
